# revision 31
# baseline (speedup 1.0000x reference)
"""Trainium2 Bass kernel for nn_Attention_23424751632639.

Computation (per (b,h)):  out = tril_strict(rope(Q) @ rope(Q).T / sqrt(N)) @ V
Chunked linear attention (exact reordering of the sums), chunk = 128 rows:
  out_c = QR_c @ M_{c-1}  +  strict_mask(QR_c @ QR_c^T) @ V_c
  M_c   = M_{c-1} + QR_c^T @ V_c          (M = running [64,64] state, PSUM)

Implementation (v4):
  * fp16 everywhere on device; all matmul accumulation stays fp32 in PSUM.
  * RoPE applied on the host; device receives QR in natural [t, n] and
    transposed [n, t] layouts plus V (scale folded into the rope tables).
  * PE p-state ramp: the TensorE only reaches its 2.4 GHz p-state after a
    multi-us gapless run of matmuls; without it every matmul runs at the
    1.2 GHz mid state.  A warmup burst of dummy 64-col matmuls on a
    memset scratch tile runs during the input-DMA prologue so real work
    starts (and stays) at full clock.
  * All input DMA is issued upfront (first chunk's slices first) across
    the sync and scalar queues; concurrent DMA does not contend with the
    PE, so there is no windowed prefetch.
  * Per chunk (4 heads): 4 state matmuls, 4x (S + inter sharing the qrt
    stationary), 4 intra matmuls lagged 2 chunks so the strict-mask
    product (DVE/ACT/GpSimd) never stalls the PE.
  * PSUM->SBUF crossings (P-mask, M snapshot, output copy) rotate across
    DVE / ACT / GpSimd.

Sharding: B*H = 32 (b,h) pairs -> 4 per core across 8 cores; no collectives.
"""

import math
import sys

import numpy as np

if "/opt/trn_rl_repo" not in sys.path:
    sys.path.insert(0, "/opt/trn_rl_repo")

B, H, T, N = 2, 16, 4096, 64
THETA = 2.0 ** 16
NCORES = 8
HPC = (B * H) // NCORES   # heads per core
CH = T // 128             # chunks per head (32)
NW = 4                    # layout windows (DMA slicing granularity)
CPW = CH // NW            # chunks per window (8)
WCOLS = CPW * HPC * N     # columns per (window, stream) slice (2048)
NWARM = 72                # p-state warmup matmuls
FILLC = 14                # chunks that get dummy-MM filler (DMA pacing)


def build_program():
    import concourse.mybir as mybir
    import concourse.tile as tile
    from concourse import bacc

    f32 = mybir.dt.float32
    f16 = mybir.dt.float16

    nc = bacc.Bacc(None, target_bir_lowering=False)
    # qn: [p, c, s, h, n]; s: 0=qr 1=v   (chunk-major natural layouts:
    # any chunk range is a contiguous per-partition DMA run)
    qn = nc.dram_tensor("qn", [128, CH * 512], f16, kind="ExternalInput")
    # qt: [p(n), c, h, t]                (transposed rope(Q))
    qt = nc.dram_tensor("qt", [64, CH * 512], f16, kind="ExternalInput")
    cst = nc.dram_tensor("cst", [128, 512], f16, kind="ExternalInput")
    # o: [p, c, h, n]
    o = nc.dram_tensor("o", [128, CH * 256], f16, kind="ExternalOutput")

    with tile.TileContext(nc) as tc:
        with (
            tc.tile_pool(name="big", bufs=1) as bigp,
            tc.tile_pool(name="mb", bufs=2) as mbp,
            tc.tile_pool(name="psb", bufs=4) as psbp,
            tc.tile_pool(name="ost", bufs=3) as ostp,
            tc.tile_pool(name="spps", bufs=3, space="PSUM") as spp,
            tc.tile_pool(name="outps", bufs=3, space="PSUM") as outp,
            tc.tile_pool(name="mps", bufs=1, space="PSUM") as mpp,
        ):
            qn_sb = bigp.tile([128, NW * 2 * WCOLS], f16)
            qt_sb = bigp.tile([64, NW * 2 * WCOLS], f16)
            cst_sb = bigp.tile([128, 512], f16)
            warm_sb = bigp.tile([128, 128], f16)
            mask4 = cst_sb[:, 0:512]

            def dma_qn(clo, chi):
                a, b = 512 * clo, 512 * chi
                nc.sync.dma_start(qn_sb[:, a:b], qn[:, a:b])

            def dma_qt(clo, chi):
                a, b = 512 * clo, 512 * chi
                nc.sync.dma_start(qt_sb[:, a:b], qt[:, a:b])

            # ---- p-state warmup: gapless dummy matmuls, no DMA deps.
            # The TensorE drops to the 1.2 GHz mid p-state after any >~1us
            # idle and does not recover, so a warm PSUM tile also provides
            # dummy-MM filler between early chunks while the input DMA
            # stream catches up.
            nc.gpsimd.memset(warm_sb[:], 0.0)
            mreg = mpp.tile([64, 256], f32, name="mreg")
            wps = mpp.tile([64, 256], f32, name="wps")

            def dummy(k):
                for _ in range(k):
                    nc.tensor.matmul(
                        wps[:, 0:64], warm_sb[:, 0:64], warm_sb[:, 64:128],
                        start=True, stop=True, skip_group_check=True)

            dummy(NWARM)

            # ---- input DMA: everything upfront on the SYNC queue only.
            # A single queue gets the full 16-engine width; chunk-major
            # layouts make every piece a contiguous per-partition run
            # (small runs pay a large per-descriptor overhead, so pieces
            # grow once the pipeline is primed).  The scalar queue stays
            # free for the ACT engine's copies; outputs go on the gpsimd
            # queue so they never sit behind the input backlog.
            dma_qt(0, 2)
            dma_qn(0, 2)
            nc.sync.dma_start(cst_sb[:], cst[:])
            dma_qt(2, 4)
            dma_qn(2, 4)
            for g in range(4, CH, 4):
                dma_qt(g, g + 4)
                dma_qn(g, g + 4)

            # per-chunk records for the 2-chunk-lagged intra
            rec = {}

            def body(c):
                k = c % 2

                def qr_sl(h):  # [128, 64] natural rope(Q) chunk
                    off = 512 * c + 64 * h
                    return qn_sb[:, off:off + 64]

                def v_sl(h):   # [128, 64] V chunk
                    off = 512 * c + 256 + 64 * h
                    return qn_sb[:, off:off + 64]

                def qrt_sl(h):  # [64, 128] transposed rope(Q) chunk
                    off = 512 * c + 128 * h
                    return qt_sb[:, off:off + 128]

                # state: M_h += QR_c^T V_c   (PSUM accumulate across chunks)
                for h in range(HPC):
                    nc.tensor.matmul(
                        mreg[:, 64 * h:64 * h + 64],
                        qr_sl(h), v_sl(h),
                        start=(c == 0 and h == 0),
                        stop=(c == CH - 1 and h == HPC - 1),
                        skip_group_check=True,
                    )

                # M snapshot for inter of chunk c+1 (always ACT: DVE carries
                # the mask products + output copies)
                mb = None
                if c < CH - 1:
                    mb = mbp.tile([64, 256], f16, tag="mb")
                    nc.scalar.copy(mb[:], mreg[:])

                # output PSUM tile per pair
                if k == 0:
                    op = outp.tile([128, 512], f32, tag="outp")
                else:
                    op = rec[c - 1]["op"]

                # S blocks (+ inter sharing the same stationary operand)
                sp = spp.tile([128, 512], f32, tag="sp")
                for h in range(HPC):
                    qrt_c = qrt_sl(h)
                    nc.tensor.matmul(
                        sp[:, 128 * h:128 * h + 128], qrt_c, qrt_c,
                        start=(h == 0), stop=(h == HPC - 1),
                    )
                    if c > 0:
                        # first write of this pair's outp zero region gets
                        # start=True (inter of even chunk; chunk 1 for pair 0)
                        nc.tensor.matmul(
                            op[:, 256 * k + 64 * h:256 * k + 64 * h + 64],
                            qrt_c, rec[c - 1]["mb"][:, 64 * h:64 * h + 64],
                            start=(h == 0 and (k == 0 or c == 1)),
                            stop=False,
                        )

                # P = S * strict-upper mask  (psum f32 -> sbuf fp16, DVE)
                psb = psbp.tile([128, 512], f16, tag="psb")
                nc.vector.tensor_mul(psb[:], sp[:], mask4)

                # intra lagged by 2 chunks so the mask never stalls the PE
                if c > 1:
                    intra(c - 2)

                # keep the PE fed wherever the input stream is tight: any
                # >~1us idle drops the TensorE to its 1.2 GHz p-state
                if c < 4:
                    dummy(8)
                elif c < 28:
                    dummy(4)

                rec[c] = {"mb": mb, "psb": psb, "op": op,
                          "v": [v_sl(h) for h in range(HPC)]}
                rec.pop(c - 3, None)

            def intra(c):
                k = c % 2
                r = rec[c]
                for h in range(HPC):
                    nc.tensor.matmul(
                        r["op"][:, 256 * k + 64 * h:256 * k + 64 * h + 64],
                        r["psb"][:, 128 * h:128 * h + 128], r["v"][h],
                        start=False, stop=(k == 1 and h == HPC - 1),
                    )
                if k == 1:
                    # pair finished: fp16 staging copy (DVE) + output DMA
                    ost = ostp.tile([128, 512], f16, tag="ost")
                    nc.vector.tensor_copy(ost[:], r["op"][:])
                    off = 256 * (c - 1)
                    nc.gpsimd.dma_start(o[:, off:off + 512], ost[:])

            for c in range(CH):
                body(c)
            intra(CH - 2)
            intra(CH - 1)

    nc.compile()
    return nc


_CACHE = {}


def _get_program():
    if "nc" not in _CACHE:
        _CACHE["nc"] = build_program()
    return _CACHE["nc"]


def _tables():
    n = np.arange(N, dtype=np.float64)
    tq = np.floor(n / 2.0) * 2.0
    freqs = 1.0 / (THETA ** (tq / N)) / (2.0 * math.pi)
    t = np.arange(T, dtype=np.float64)[:, None]
    ang = ((t * freqs[None, :]) % 1.0) * (2.0 * math.pi)
    scale = float(N) ** -0.25
    cc = (np.cos(ang) * scale).astype(np.float32)
    ss = (np.sin(ang) * scale).astype(np.float32)
    ss[:, 0::2] *= -1.0
    return cc, ss


def make_inputs(Q, V):
    """Full inputs -> list of per-core {'qn','qt','cst'} fp16 host arrays."""
    Q = np.asarray(Q, dtype=np.float32).reshape(NCORES, HPC, T, N)
    V = np.asarray(V, dtype=np.float32).reshape(NCORES, HPC, T, N)
    cc, ss = _tables()
    sq = np.empty_like(Q)
    sq[..., 0::2] = Q[..., 1::2]
    sq[..., 1::2] = Q[..., 0::2]
    qr = (Q * cc + sq * ss).astype(np.float16)  # scaled rope(Q)
    v16 = V.astype(np.float16)

    # natural: [core, h, c, p, n] -> [core, p, c, s, h, n]
    def nat(x):
        x = x.reshape(NCORES, HPC, CH, 128, N)
        return np.transpose(x, (0, 3, 2, 1, 4))  # core p c h n

    qn_h = np.stack([nat(qr), nat(v16)], axis=3)  # core p c s h n
    qn_h = np.ascontiguousarray(qn_h.reshape(NCORES, 128, CH * 512))

    # transposed: [core, n, c, h, t]
    qt_h = qr.reshape(NCORES, HPC, CH, 128, N)
    qt_h = np.transpose(qt_h, (0, 4, 2, 1, 3))  # core n c h t
    qt_h = np.ascontiguousarray(qt_h.reshape(NCORES, 64, CH * 512))

    mu = np.triu(np.ones((128, 128), dtype=np.float16), k=1)
    cst = np.ascontiguousarray(np.concatenate([mu] * 4, axis=1))  # [128, 512]
    return [{"qn": qn_h[i], "qt": qt_h[i], "cst": cst}
            for i in range(NCORES)]


def unpack_out(results):
    """list of per-core {'o': [128, NW*WCOLS] fp16} -> [B,H,T,N] f32."""
    o = np.stack([r["o"] for r in results], axis=0)
    o = o.reshape(NCORES, 128, CH, HPC, N)
    o = np.transpose(o, (0, 3, 2, 1, 4))  # [8, HPC, CH, 128, N]
    return np.ascontiguousarray(
        o.reshape(B, H, T, N).astype(np.float32))


def kernel(Q, V):
    from concourse.bass_utils import run_bass_kernel_spmd

    nc = _get_program()
    in_maps = make_inputs(Q, V)
    res = run_bass_kernel_spmd(nc, in_maps, core_ids=list(range(NCORES)))
    return unpack_out(res.results)


# revision 33
# speedup vs baseline: 1.0863x; 1.0863x over previous
"""Trainium2 Bass kernel for nn_Attention_23424751632639.

Computation (per (b,h)):  out = tril_strict(rope(Q) @ rope(Q).T / sqrt(N)) @ V
Chunked linear attention (exact reordering of the sums), chunk = 128 rows:
  out_c = QR_c @ M_{c-1}  +  strict_mask(QR_c @ QR_c^T) @ V_c
  M_c   = M_{c-1} + QR_c^T @ V_c          (M = running [64,64] state, PSUM)

Implementation (v4):
  * fp16 everywhere on device; all matmul accumulation stays fp32 in PSUM.
  * RoPE applied on the host; device receives QR in natural [t, n] and
    transposed [n, t] layouts plus V (scale folded into the rope tables).
  * PE p-state ramp: the TensorE only reaches its 2.4 GHz p-state after a
    multi-us gapless run of matmuls; without it every matmul runs at the
    1.2 GHz mid state.  A warmup burst of dummy 64-col matmuls on a
    memset scratch tile runs during the input-DMA prologue so real work
    starts (and stays) at full clock.
  * All input DMA is issued upfront (first chunk's slices first) across
    the sync and scalar queues; concurrent DMA does not contend with the
    PE, so there is no windowed prefetch.
  * Per chunk (4 heads): 4 state matmuls, 4x (S + inter sharing the qrt
    stationary), 4 intra matmuls lagged 2 chunks so the strict-mask
    product (DVE/ACT/GpSimd) never stalls the PE.
  * PSUM->SBUF crossings (P-mask, M snapshot, output copy) rotate across
    DVE / ACT / GpSimd.

Sharding: B*H = 32 (b,h) pairs -> 4 per core across 8 cores; no collectives.
"""

import math
import sys

import numpy as np

if "/opt/trn_rl_repo" not in sys.path:
    sys.path.insert(0, "/opt/trn_rl_repo")

B, H, T, N = 2, 16, 4096, 64
THETA = 2.0 ** 16
NCORES = 8
HPC = (B * H) // NCORES   # heads per core
CH = T // 128             # chunks per head (32)
NW = 4                    # layout windows (DMA slicing granularity)
CPW = CH // NW            # chunks per window (8)
WCOLS = CPW * HPC * N     # columns per (window, stream) slice (2048)
NWARM = 72                # p-state warmup matmuls
FILLC = 14                # chunks that get dummy-MM filler (DMA pacing)


def build_program():
    import concourse.mybir as mybir
    import concourse.tile as tile
    from concourse import bacc

    f32 = mybir.dt.float32
    f16 = mybir.dt.float16

    nc = bacc.Bacc(None, target_bir_lowering=False)
    # qn: [p, c, s, h, n]; s: 0=qr 1=v   (chunk-major natural layouts:
    # any chunk range is a contiguous per-partition DMA run)
    qn = nc.dram_tensor("qn", [128, CH * 512], f16, kind="ExternalInput")
    # qt: [p(n), c, h, t]                (transposed rope(Q))
    qt = nc.dram_tensor("qt", [64, CH * 512], f16, kind="ExternalInput")
    cst = nc.dram_tensor("cst", [128, 512], f16, kind="ExternalInput")
    # o: [p, c, h, n]
    o = nc.dram_tensor("o", [128, CH * 256], f16, kind="ExternalOutput")

    with tile.TileContext(nc) as tc:
        with (
            tc.tile_pool(name="big", bufs=1) as bigp,
            tc.tile_pool(name="mb", bufs=2) as mbp,
            tc.tile_pool(name="psb", bufs=4) as psbp,
            tc.tile_pool(name="ost", bufs=3) as ostp,
            tc.tile_pool(name="spps", bufs=3, space="PSUM") as spp,
            tc.tile_pool(name="outps", bufs=3, space="PSUM") as outp,
            tc.tile_pool(name="mps", bufs=1, space="PSUM") as mpp,
        ):
            qn_sb = bigp.tile([128, NW * 2 * WCOLS], f16)
            qt_sb = bigp.tile([64, NW * 2 * WCOLS], f16)
            cst_sb = bigp.tile([128, 512], f16)
            warm_sb = bigp.tile([128, 128], f16)
            mask4 = cst_sb[:, 0:512]

            def dma_qn(clo, chi):
                a, b = 512 * clo, 512 * chi
                nc.sync.dma_start(qn_sb[:, a:b], qn[:, a:b])

            def dma_qt(clo, chi):
                a, b = 512 * clo, 512 * chi
                nc.sync.dma_start(qt_sb[:, a:b], qt[:, a:b])

            # ---- p-state warmup: gapless dummy matmuls, no DMA deps.
            # The TensorE drops to the 1.2 GHz mid p-state after any >~1us
            # idle and does not recover, so a warm PSUM tile also provides
            # dummy-MM filler between early chunks while the input DMA
            # stream catches up.
            nc.gpsimd.memset(warm_sb[:], 0.0)
            mreg = mpp.tile([64, 256], f32, name="mreg")
            wps = mpp.tile([64, 256], f32, name="wps")

            def dummy(k):
                for _ in range(k):
                    nc.tensor.matmul(
                        wps[:, 0:64], warm_sb[:, 0:64], warm_sb[:, 64:128],
                        start=True, stop=True, skip_group_check=True)

            dummy(NWARM)

            # ---- input DMA: everything upfront on the SYNC queue only.
            # A single queue gets the full 16-engine width; chunk-major
            # layouts make every piece a contiguous per-partition run
            # (small runs pay a large per-descriptor overhead, so pieces
            # grow once the pipeline is primed).  The scalar queue stays
            # free for the ACT engine's copies; outputs go on the gpsimd
            # queue so they never sit behind the input backlog.
            dma_qt(0, 2)
            dma_qn(0, 2)
            nc.sync.dma_start(cst_sb[:], cst[:])
            dma_qt(2, 4)
            dma_qn(2, 4)
            dma_qt(4, 8)
            dma_qn(4, 8)
            dma_qt(8, 12)
            dma_qn(8, 12)
            dma_qt(12, 16)
            dma_qn(12, 16)
            dma_qt(16, 24)
            dma_qn(16, 24)
            dma_qt(24, 32)
            dma_qn(24, 32)

            # per-chunk records for the 2-chunk-lagged intra
            rec = {}

            def body(c):
                k = c % 2

                def qr_sl(h):  # [128, 64] natural rope(Q) chunk
                    off = 512 * c + 64 * h
                    return qn_sb[:, off:off + 64]

                def v_sl(h):   # [128, 64] V chunk
                    off = 512 * c + 256 + 64 * h
                    return qn_sb[:, off:off + 64]

                def qrt_sl(h):  # [64, 128] transposed rope(Q) chunk
                    off = 512 * c + 128 * h
                    return qt_sb[:, off:off + 128]

                # state: M_h += QR_c^T V_c   (PSUM accumulate across chunks)
                for h in range(HPC):
                    nc.tensor.matmul(
                        mreg[:, 64 * h:64 * h + 64],
                        qr_sl(h), v_sl(h),
                        start=(c == 0 and h == 0),
                        stop=(c == CH - 1 and h == HPC - 1),
                        skip_group_check=True,
                    )

                # M snapshot for inter of chunk c+1 (always ACT: DVE carries
                # the mask products + output copies)
                mb = None
                if c < CH - 1:
                    mb = mbp.tile([64, 256], f16, tag="mb")
                    nc.scalar.copy(mb[:], mreg[:])

                # output PSUM tile per pair
                if k == 0:
                    op = outp.tile([128, 512], f32, tag="outp")
                else:
                    op = rec[c - 1]["op"]

                # S blocks (+ inter sharing the same stationary operand)
                sp = spp.tile([128, 512], f32, tag="sp")
                for h in range(HPC):
                    qrt_c = qrt_sl(h)
                    nc.tensor.matmul(
                        sp[:, 128 * h:128 * h + 128], qrt_c, qrt_c,
                        start=(h == 0), stop=(h == HPC - 1),
                    )
                    if c > 0:
                        # first write of this pair's outp zero region gets
                        # start=True (inter of even chunk; chunk 1 for pair 0)
                        nc.tensor.matmul(
                            op[:, 256 * k + 64 * h:256 * k + 64 * h + 64],
                            qrt_c, rec[c - 1]["mb"][:, 64 * h:64 * h + 64],
                            start=(h == 0 and (k == 0 or c == 1)),
                            stop=False,
                        )

                # P = S * strict-upper mask  (psum f32 -> sbuf fp16, DVE)
                psb = psbp.tile([128, 512], f16, tag="psb")
                nc.vector.tensor_mul(psb[:], sp[:], mask4)

                # intra lagged by 2 chunks so the mask never stalls the PE
                if c > 1:
                    intra(c - 2)

                # keep the PE fed wherever the input stream is tight: any
                # >~1us idle drops the TensorE to its 1.2 GHz p-state
                if c < 16:
                    dummy(8)
                elif c < 24:
                    dummy(2)

                rec[c] = {"mb": mb, "psb": psb, "op": op,
                          "v": [v_sl(h) for h in range(HPC)]}
                rec.pop(c - 3, None)

            def intra(c):
                k = c % 2
                r = rec[c]
                for h in range(HPC):
                    nc.tensor.matmul(
                        r["op"][:, 256 * k + 64 * h:256 * k + 64 * h + 64],
                        r["psb"][:, 128 * h:128 * h + 128], r["v"][h],
                        start=False, stop=(k == 1 and h == HPC - 1),
                    )
                if k == 1:
                    # pair finished: fp16 staging copy (DVE) + output DMA
                    ost = ostp.tile([128, 512], f16, tag="ost")
                    nc.vector.tensor_copy(ost[:], r["op"][:])
                    off = 256 * (c - 1)
                    nc.gpsimd.dma_start(o[:, off:off + 512], ost[:])

            for c in range(CH):
                body(c)
            intra(CH - 2)
            intra(CH - 1)

    nc.compile()
    return nc


_CACHE = {}


def _get_program():
    if "nc" not in _CACHE:
        _CACHE["nc"] = build_program()
    return _CACHE["nc"]


def _tables():
    n = np.arange(N, dtype=np.float64)
    tq = np.floor(n / 2.0) * 2.0
    freqs = 1.0 / (THETA ** (tq / N)) / (2.0 * math.pi)
    t = np.arange(T, dtype=np.float64)[:, None]
    ang = ((t * freqs[None, :]) % 1.0) * (2.0 * math.pi)
    scale = float(N) ** -0.25
    cc = (np.cos(ang) * scale).astype(np.float32)
    ss = (np.sin(ang) * scale).astype(np.float32)
    ss[:, 0::2] *= -1.0
    return cc, ss


def make_inputs(Q, V):
    """Full inputs -> list of per-core {'qn','qt','cst'} fp16 host arrays."""
    Q = np.asarray(Q, dtype=np.float32).reshape(NCORES, HPC, T, N)
    V = np.asarray(V, dtype=np.float32).reshape(NCORES, HPC, T, N)
    cc, ss = _tables()
    sq = np.empty_like(Q)
    sq[..., 0::2] = Q[..., 1::2]
    sq[..., 1::2] = Q[..., 0::2]
    qr = (Q * cc + sq * ss).astype(np.float16)  # scaled rope(Q)
    v16 = V.astype(np.float16)

    # natural: [core, h, c, p, n] -> [core, p, c, s, h, n]
    def nat(x):
        x = x.reshape(NCORES, HPC, CH, 128, N)
        return np.transpose(x, (0, 3, 2, 1, 4))  # core p c h n

    qn_h = np.stack([nat(qr), nat(v16)], axis=3)  # core p c s h n
    qn_h = np.ascontiguousarray(qn_h.reshape(NCORES, 128, CH * 512))

    # transposed: [core, n, c, h, t]
    qt_h = qr.reshape(NCORES, HPC, CH, 128, N)
    qt_h = np.transpose(qt_h, (0, 4, 2, 1, 3))  # core n c h t
    qt_h = np.ascontiguousarray(qt_h.reshape(NCORES, 64, CH * 512))

    mu = np.triu(np.ones((128, 128), dtype=np.float16), k=1)
    cst = np.ascontiguousarray(np.concatenate([mu] * 4, axis=1))  # [128, 512]
    return [{"qn": qn_h[i], "qt": qt_h[i], "cst": cst}
            for i in range(NCORES)]


def unpack_out(results):
    """list of per-core {'o': [128, NW*WCOLS] fp16} -> [B,H,T,N] f32."""
    o = np.stack([r["o"] for r in results], axis=0)
    o = o.reshape(NCORES, 128, CH, HPC, N)
    o = np.transpose(o, (0, 3, 2, 1, 4))  # [8, HPC, CH, 128, N]
    return np.ascontiguousarray(
        o.reshape(B, H, T, N).astype(np.float32))


def kernel(Q, V):
    from concourse.bass_utils import run_bass_kernel_spmd

    nc = _get_program()
    in_maps = make_inputs(Q, V)
    res = run_bass_kernel_spmd(nc, in_maps, core_ids=list(range(NCORES)))
    return unpack_out(res.results)
